# revision 12
# baseline (speedup 1.0000x reference)
"""Trainium2 Bass kernel for the MANN (memory-augmented NN) problem.

Reference computation (per batch of B=2048 samples):
    h        = tanh(x @ W_h + b_h)                  [B, 512]
    gate     = x @ W_g + b_g                        [B, 1]
    read_key = tanh(h @ W_k + b_k)                  [B, 64]
    kn       = read_key / (||read_key|| + eps)
    Mn       = M / (||M_row|| + eps)                [65536, 64]
    sim      = kn @ Mn.T                            [B, 65536]
    w_read   = softmax(sim, axis=-1)
    r        = w_read @ M                           [B, 64]
    out      = (concat(h, r) @ W_out + b_out)[:, 0] [B]
    returns (out, h[-1], gate[-1], w_read[-1])

Strategy: data-parallel over batch across 8 NeuronCores (256 samples each).
On each core everything is computed in a "transposed" layout (features on
partitions, batch on the free axis) so the streaming pass over the 65536
memory locations needs NO on-chip transposes:

    simT_chunk [128 locs, 256 B] = matmul(lhsT=MnT_chunk [64, 128] bf16,
                                          rhs =knT       [64, 256] bf16)
    w2 = exp(simT)                     (scalar engine, f32 PSUM -> bf16 SBUF)
    rT [65, 256] += matmul(lhsT=[M_chunk | ones] [128, 65] bf16,
                           rhs =w2 [128, 256] bf16)

The ones column folded into the r-matmul accumulates the softmax
denominator Z in row 64 of the same PSUM accumulator.  Cosine similarity
is bounded in [-1, 1] so exp() needs no running-max for stability.

The read-key/knT needed by the streaming loop is computed via a fast bf16
controller pass (~8us) so the memory stream starts early; an fp32
controller pass (for the returned h[-1] and the h @ W_out part of the
output) runs later inside the PE slack of the exp-bound main loop.
"""

import numpy as np

NCORES = 8
B = 2048
IN_DIM = 512
CTRL = 512
N_LOC = 65536
LOC = 64
EPS = 1e-8

BC = B // NCORES          # 256 batch per core
KT = IN_DIM // 128        # 4 input k-tiles
CT = CTRL // 128          # 4 ctrl tiles
NCHUNK = N_LOC // 128     # 512 location chunks
SUPER = 16                # chunks per DMA superblock
NSB = NCHUNK // SUPER     # 32 superblocks
GROUPS = [6, 6, 4]        # exp batching: first two on ScalarE, last on VectorE
DVE_EXP = True            # paired-Schraudolph exp on the Vector engine
LOG2E = 1.4426950408889634
SCH_A = 0.5 * LOG2E * (1 << 23)
SCH_C = 450000.0
SCH_B1 = 127.0 * (1 << 23) - (1 << 21) - SCH_C
SCH_B2 = 127.0 * (1 << 23) + (1 << 21) - SCH_C

_CACHE = {}
DEBUG_TAPS = False


def _build_program():
    import concourse.mybir as mybir
    import concourse.tile as tile
    from concourse import bacc
    from contextlib import ExitStack

    f32 = mybir.dt.float32
    bf16 = mybir.dt.bfloat16
    i32 = mybir.dt.int32
    AF = mybir.ActivationFunctionType
    AOT = mybir.AluOpType

    nc = bacc.Bacc("TRN2", target_bir_lowering=False, debug=False)

    # ---------------- DRAM I/O (per-core shapes; host pre-swizzled) ----------
    xTb = nc.dram_tensor("xTb", [128, KT, BC], bf16, kind="ExternalInput")
    xTl = nc.dram_tensor("xTl", [128, KT, BC], bf16, kind="ExternalInput")
    Whb = nc.dram_tensor("Whb", [128, KT, CTRL], bf16, kind="ExternalInput")
    Whl = nc.dram_tensor("Whl", [128, KT, CTRL], bf16, kind="ExternalInput")
    bh = nc.dram_tensor("bh", [128, CT], f32, kind="ExternalInput")
    Wgb = nc.dram_tensor("Wgb", [128, KT], bf16, kind="ExternalInput")
    Wgl = nc.dram_tensor("Wgl", [128, KT], bf16, kind="ExternalInput")
    bg = nc.dram_tensor("bg", [1, 1], f32, kind="ExternalInput")
    Wkb = nc.dram_tensor("Wkb", [128, KT, LOC], bf16, kind="ExternalInput")
    bk = nc.dram_tensor("bk", [LOC, 1], f32, kind="ExternalInput")
    MnT = nc.dram_tensor("MnT", [LOC, N_LOC], bf16, kind="ExternalInput")
    Mo = nc.dram_tensor("Mo", [128, NCHUNK, LOC + 1], bf16, kind="ExternalInput")
    WoH = nc.dram_tensor("WoH", [128, CT], f32, kind="ExternalInput")
    WoR = nc.dram_tensor("WoR", [LOC, 1], f32, kind="ExternalInput")
    bo = nc.dram_tensor("bo", [1, 1], f32, kind="ExternalInput")

    out_d = nc.dram_tensor("out", [1, BC], f32, kind="ExternalOutput")
    hl_d = nc.dram_tensor("hl", [128, CT], f32, kind="ExternalOutput")
    gt_d = nc.dram_tensor("gt", [1, BC], f32, kind="ExternalOutput")
    wl_d = nc.dram_tensor("wl", [128, NCHUNK], f32, kind="ExternalOutput")
    if DEBUG_TAPS:
        dbg_kn = nc.dram_tensor("dbg_kn", [LOC, BC], f32, kind="ExternalOutput")
        dbg_racc = nc.dram_tensor("dbg_racc", [LOC + 1, BC], f32,
                                  kind="ExternalOutput")

    with tile.TileContext(nc) as tc, ExitStack() as ctx:
        const = ctx.enter_context(tc.tile_pool(name="const", bufs=1))
        mnp = ctx.enter_context(tc.tile_pool(name="mnp", bufs=3))
        mop = ctx.enter_context(tc.tile_pool(name="mop", bufs=3))
        w2p = ctx.enter_context(tc.tile_pool(name="w2p", bufs=3))
        ps_sim = ctx.enter_context(tc.tile_pool(name="ps_sim", bufs=2, space="PSUM"))
        ps_r = ctx.enter_context(tc.tile_pool(name="ps_r", bufs=1, space="PSUM"))
        ps_misc = ctx.enter_context(tc.tile_pool(name="ps_misc", bufs=1, space="PSUM"))

        def tap(dram, ap, shape):
            t = const.tile(shape, f32, tag="tap" + dram.name)
            nc.vector.tensor_copy(out=t[:], in_=ap)
            nc.sync.dma_start(out=dram[:], in_=t[:])

        # ------------- load weights/inputs -------------
        xTb_sb = const.tile([128, KT, BC], bf16)
        nc.sync.dma_start(out=xTb_sb[:], in_=xTb[:])
        xTl_sb = const.tile([128, KT, BC], bf16)
        nc.sync.dma_start(out=xTl_sb[:], in_=xTl[:])
        Whb_sb = const.tile([128, KT, CTRL], bf16)
        nc.sync.dma_start(out=Whb_sb[:], in_=Whb[:])
        Whl_sb = const.tile([128, KT, CTRL], bf16)
        nc.sync.dma_start(out=Whl_sb[:], in_=Whl[:])
        bh_sb = const.tile([128, CT], f32)
        nc.sync.dma_start(out=bh_sb[:], in_=bh[:])
        Wgb_sb = const.tile([128, KT], bf16)
        nc.sync.dma_start(out=Wgb_sb[:], in_=Wgb[:])
        Wgl_sb = const.tile([128, KT], bf16)
        nc.sync.dma_start(out=Wgl_sb[:], in_=Wgl[:])
        bg_sb = const.tile([1, 1], f32)
        nc.sync.dma_start(out=bg_sb[:], in_=bg[:])
        Wkb_sb = const.tile([128, KT, LOC], bf16)
        nc.sync.dma_start(out=Wkb_sb[:], in_=Wkb[:])
        bk_sb = const.tile([LOC, 1], f32)
        nc.sync.dma_start(out=bk_sb[:], in_=bk[:])

        ones_sb = const.tile([128, 128], f32)
        nc.vector.memset(ones_sb[:], 1.0)

        # ------------- controller: h via bf16x2 (hi/lo) => ~fp32 accuracy ---
        hT_sb = const.tile([128, CT, BC], f32)     # for h[-1] export + out_h
        hTb_sb = const.tile([128, CT, BC], bf16)   # for the read-key matmul
        for ct in range(CT):
            ps_h = ps_sim.tile([128, BC], f32, tag="s")
            n = 0
            for k in range(KT):
                cs = slice(ct * 128, (ct + 1) * 128)
                for lhs, rhs in ((Whb_sb[:, k, cs], xTb_sb[:, k, :]),
                                 (Whl_sb[:, k, cs], xTb_sb[:, k, :]),
                                 (Whb_sb[:, k, cs], xTl_sb[:, k, :])):
                    nc.tensor.matmul(ps_h[:], lhs, rhs, start=(n == 0),
                                     stop=(n == 3 * KT - 1))
                    n += 1
            nc.scalar.activation(out=hT_sb[:, ct, :], in_=ps_h[:], func=AF.Tanh,
                                 bias=bh_sb[:, ct:ct + 1], scale=1.0)
            nc.scalar.activation(out=hTb_sb[:, ct, :], in_=ps_h[:], func=AF.Tanh,
                                 bias=bh_sb[:, ct:ct + 1], scale=1.0)

        # gate = x W_g + b_g (bf16x2)
        ps_g = ps_sim.tile([1, BC], f32, tag="s")
        n = 0
        for k in range(KT):
            for lhs, rhs in ((Wgb_sb[:, k:k + 1], xTb_sb[:, k, :]),
                             (Wgl_sb[:, k:k + 1], xTb_sb[:, k, :]),
                             (Wgb_sb[:, k:k + 1], xTl_sb[:, k, :])):
                nc.tensor.matmul(ps_g[:], lhs, rhs, start=(n == 0),
                                 stop=(n == 3 * KT - 1))
                n += 1
        gt_sb = const.tile([1, BC], f32)
        nc.scalar.activation(out=gt_sb[:], in_=ps_g[:], func=AF.Identity,
                             bias=bg_sb[0:1, 0:1], scale=1.0)
        nc.sync.dma_start(out=gt_d[:], in_=gt_sb[:])

        # h[-1]: column BC-1 of hT
        hl_sb = const.tile([128, CT], f32)
        nc.vector.tensor_copy(out=hl_sb[:], in_=hT_sb[:, :, BC - 1])
        nc.sync.dma_start(out=hl_d[:], in_=hl_sb[:])

        ps_rk = ps_sim.tile([LOC, BC], f32, tag="s")
        for k in range(KT):
            nc.tensor.matmul(ps_rk[:], Wkb_sb[:, k, :], hTb_sb[:, k, :],
                             start=(k == 0), stop=(k == KT - 1))
        rkT_sb = const.tile([LOC, BC], bf16)
        nc.scalar.activation(out=rkT_sb[:], in_=ps_rk[:], func=AF.Tanh,
                             bias=bk_sb[:], scale=1.0)

        # kn = rk / ||rk||: sum of squares via ones-matmul, rsqrt via ln/exp
        rksq_sb = const.tile([LOC, BC], f32)
        nc.vector.tensor_mul(rksq_sb[:], rkT_sb[:], rkT_sb[:])
        ps_ss = ps_sim.tile([1, BC], f32, tag="s")
        nc.tensor.matmul(ps_ss[:], ones_sb[0:LOC, 0:1], rksq_sb[:],
                         start=True, stop=True)
        lnss_sb = const.tile([1, BC], f32)
        nc.scalar.activation(out=lnss_sb[:], in_=ps_ss[:], func=AF.Ln)
        invn_sb = const.tile([1, BC], f32)
        nc.scalar.activation(out=invn_sb[:], in_=lnss_sb[:], func=AF.Exp,
                             scale=-0.5)
        ps_bc = ps_sim.tile([LOC, BC], f32, tag="s")
        nc.tensor.matmul(ps_bc[:], ones_sb[0:1, 0:LOC], invn_sb[:],
                         start=True, stop=True)
        knT_sb = const.tile([LOC, BC], bf16)
        nc.vector.tensor_mul(knT_sb[:], rkT_sb[:], ps_bc[:])
        if DEBUG_TAPS:
            tap(dbg_kn, knT_sb[:], [LOC, BC])

        # ------------- streaming pass over the 65536 memory locations -------
        wlast_sb = const.tile([128, NCHUNK], f32)   # unnormalized w of sample BC-1
        ps_rT = ps_r.tile([LOC + 1, BC], f32)       # rows 0..63: r^T; row 64: Z
        for sb in range(NSB):
            c0 = sb * SUPER
            mn_t = mnp.tile([LOC, SUPER, 128], bf16)
            nc.sync.dma_start(
                out=mn_t[:],
                in_=MnT[:, c0 * 128:(c0 + SUPER) * 128].rearrange(
                    "f (s c) -> f s c", s=SUPER),
            )
            mo_t = mop.tile([128, SUPER, LOC + 1], bf16)
            nc.sync.dma_start(out=mo_t[:], in_=Mo[:, c0:c0 + SUPER, :])
            w2_t = w2p.tile([128, SUPER, BC], bf16)
            g0 = 0
            for gi, gsz in enumerate(GROUPS):
                ps_s = ps_sim.tile([128, gsz, BC], f32, tag="s")
                for j in range(gsz):
                    nc.tensor.matmul(ps_s[:, j, :], mn_t[:, g0 + j, :], knT_sb[:],
                                     start=True, stop=True)
                if DVE_EXP and gi == len(GROUPS) - 1:
                    # exp on VectorE: product of two phase-shifted Schraudolph
                    # approximations (bitcast(int32(x*A+B)) ~ 2^(x*log2e/2))
                    y1_t = w2p.tile([128, gsz, BC], i32, tag="y1")
                    nc.vector.tensor_scalar(out=y1_t[:], in0=ps_s[:],
                                            scalar1=SCH_A, scalar2=SCH_B1,
                                            op0=AOT.mult, op1=AOT.add)
                    y2_t = w2p.tile([128, gsz, BC], i32, tag="y2")
                    nc.vector.tensor_scalar(out=y2_t[:], in0=ps_s[:],
                                            scalar1=SCH_A, scalar2=SCH_B2,
                                            op0=AOT.mult, op1=AOT.add)
                    nc.vector.tensor_mul(w2_t[:, g0:g0 + gsz, :],
                                         y1_t[:].bitcast(f32),
                                         y2_t[:].bitcast(f32))
                else:
                    nc.scalar.activation(out=w2_t[:, g0:g0 + gsz, :], in_=ps_s[:],
                                         func=AF.Exp)
                g0 += gsz
            for j in range(SUPER):
                c = c0 + j
                nc.tensor.matmul(ps_rT[:], mo_t[:, j, :], w2_t[:, j, :],
                                 start=(c == 0), stop=(c == NCHUNK - 1))
            nc.gpsimd.tensor_copy(out=wlast_sb[:, c0:c0 + SUPER],
                                  in_=w2_t[:, :, BC - 1])
        if DEBUG_TAPS:
            tap(dbg_racc, ps_rT[:], [LOC + 1, BC])

        # ------------- epilogue weights ---------------------------------------
        WoH_sb = const.tile([128, CT], f32)
        nc.sync.dma_start(out=WoH_sb[:], in_=WoH[:])
        WoR_sb = const.tile([LOC, 1], f32)
        nc.sync.dma_start(out=WoR_sb[:], in_=WoR[:])
        bo_sb = const.tile([1, 1], f32)
        nc.sync.dma_start(out=bo_sb[:], in_=bo[:])

        # ------------- epilogue ---------------------------------------------
        # 1/Z on partition 64 (where Z landed), then PE-broadcasts
        zw_sb = const.tile([128, BC], f32)
        nc.vector.reciprocal(out=zw_sb[64:65, :], in_=ps_rT[LOC:LOC + 1, :])

        racc_sb = const.tile([LOC, BC], f32)
        nc.vector.tensor_copy(out=racc_sb[:], in_=ps_rT[0:LOC, :])

        # broadcast 1/Z from partition 64 to partition 0 (row 0 of [64, BC])
        ps_zb = ps_misc.tile([LOC, BC], f32, tag="m")
        nc.tensor.matmul(ps_zb[:], ones_sb[64:65, 0:LOC], zw_sb[64:65, :],
                         start=True, stop=True)
        zb_sb = const.tile([LOC, BC], f32)
        nc.vector.tensor_copy(out=zb_sb[:], in_=ps_zb[:])

        # out_r = (rT^T WoR) / Z as [1, 256]
        ps_or = ps_misc.tile([1, BC], f32, tag="m")
        nc.tensor.matmul(ps_or[:], WoR_sb[:], racc_sb[:], start=True, stop=True)
        t1_sb = const.tile([1, BC], f32)
        nc.vector.tensor_mul(t1_sb[:], zb_sb[0:1, :], ps_or[:])

        # out_h = hT^T WoH as [1, 256]; final = out_h + out_r + bo
        ps_oh = ps_misc.tile([1, BC], f32, tag="m")
        for k in range(CT):
            nc.tensor.matmul(ps_oh[:], WoH_sb[:, k:k + 1], hT_sb[:, k, :],
                             start=(k == 0), stop=(k == CT - 1))
        t2_sb = const.tile([1, BC], f32)
        nc.vector.tensor_add(t2_sb[:], t1_sb[:], ps_oh[:])
        outv_sb = const.tile([1, BC], f32)
        nc.vector.tensor_scalar_add(outv_sb[:], t2_sb[:], bo_sb[0:1, 0:1])
        nc.sync.dma_start(out=out_d[:], in_=outv_sb[:])

        # w_read[-1] = wlast * (1/Z[BC-1]) broadcast to all 128 partitions
        ps_zl = ps_misc.tile([128, 1], f32, tag="m")
        nc.tensor.matmul(ps_zl[:], ones_sb[64:65, :], zw_sb[64:65, BC - 1:BC],
                         start=True, stop=True)
        zl_sb = const.tile([128, 1], f32)
        nc.vector.tensor_copy(out=zl_sb[:], in_=ps_zl[:])
        wlf_sb = const.tile([128, NCHUNK], f32)
        nc.vector.tensor_scalar_mul(wlf_sb[:], wlast_sb[:], zl_sb[:, 0:1])
        nc.sync.dma_start(out=wl_d[:], in_=wlf_sb[:])

    nc.compile()
    return nc


def _get_program():
    if "nc" not in _CACHE:
        _CACHE["nc"] = _build_program()
    return _CACHE["nc"]


def _prep_in_maps(inputs):
    return _prep(**{k: np.asarray(v) for k, v in inputs.items()})


def _prep(x, W_h, b_h, W_g, b_g, W_k, b_k, M, W_out, b_out):
    import ml_dtypes
    bf = ml_dtypes.bfloat16

    x = np.ascontiguousarray(np.asarray(x, dtype=np.float32))
    W_h = np.asarray(W_h, dtype=np.float32)
    b_h = np.asarray(b_h, dtype=np.float32)
    W_g = np.asarray(W_g, dtype=np.float32)
    b_g = np.asarray(b_g, dtype=np.float32)
    W_k = np.asarray(W_k, dtype=np.float32)
    b_k = np.asarray(b_k, dtype=np.float32)
    M = np.ascontiguousarray(np.asarray(M, dtype=np.float32))
    W_out = np.asarray(W_out, dtype=np.float32)
    b_out = np.asarray(b_out, dtype=np.float32)

    # ---- host-side layout prep (weight swizzles for SBUF-friendly DMA) ----
    norms = np.linalg.norm(M, axis=1, keepdims=True)
    MnT = np.ascontiguousarray((M / (norms + EPS)).T).astype(bf)  # [64, 65536]
    Mo = np.concatenate([M, np.ones((N_LOC, 1), np.float32)], axis=1)
    Mo = np.ascontiguousarray(
        Mo.reshape(NCHUNK, 128, LOC + 1).transpose(1, 0, 2)).astype(bf)

    Wh_p = np.ascontiguousarray(W_h.reshape(KT, 128, CTRL).transpose(1, 0, 2))
    bh_p = np.ascontiguousarray(b_h.reshape(CT, 128).T)
    Wg_p = np.ascontiguousarray(W_g[:, 0].reshape(KT, 128).T)
    bg_p = b_g.reshape(1, 1)
    Wk_p = np.ascontiguousarray(W_k.reshape(KT, 128, LOC).transpose(1, 0, 2))
    bk_p = b_k.reshape(LOC, 1)
    WoH_p = np.ascontiguousarray(W_out[:CTRL, 0].reshape(CT, 128).T)
    WoR_p = np.ascontiguousarray(W_out[CTRL:, 0:1])
    bo_p = b_out.reshape(1, 1)

    def hilo(a):
        hi = a.astype(bf)
        lo = (a - hi.astype(np.float32)).astype(bf)
        return hi, lo

    Whb_p, Whl_p = hilo(Wh_p)
    Wgb_p, Wgl_p = hilo(Wg_p)
    shared = dict(Whb=Whb_p, Whl=Whl_p, bh=bh_p, Wgb=Wgb_p, Wgl=Wgl_p, bg=bg_p,
                  Wkb=Wk_p.astype(bf), bk=bk_p, MnT=MnT, Mo=Mo,
                  WoH=WoH_p, WoR=WoR_p, bo=bo_p)
    in_maps = []
    for c in range(NCORES):
        xc = x[c * BC:(c + 1) * BC]                              # [256, 512]
        xT_p = np.ascontiguousarray(
            xc.T.reshape(KT, 128, BC).transpose(1, 0, 2))        # [128, 4, 256]
        xb, xl = hilo(xT_p)
        in_maps.append(dict(shared, xTb=xb, xTl=xl))
    return in_maps


def kernel(x, W_h, b_h, W_g, b_g, W_k, b_k, M, W_out, b_out):
    from concourse.bass_utils import run_bass_kernel_spmd

    in_maps = _prep(x, W_h, b_h, W_g, b_g, W_k, b_k, M, W_out, b_out)
    nc = _get_program()
    res = run_bass_kernel_spmd(nc, in_maps, core_ids=list(range(NCORES))).results

    output = np.concatenate([res[c]["out"][0] for c in range(NCORES)])
    h_last = np.ascontiguousarray(res[NCORES - 1]["hl"].T).reshape(CTRL)
    gate_last = res[NCORES - 1]["gt"][0, BC - 1:BC].copy()
    w_read_last = np.ascontiguousarray(res[NCORES - 1]["wl"].T).reshape(N_LOC)
    return (output.astype(np.float32), h_last.astype(np.float32),
            gate_last.astype(np.float32), w_read_last.astype(np.float32))


# revision 13
# speedup vs baseline: 1.2165x; 1.2165x over previous
"""Trainium2 Bass kernel for the MANN (memory-augmented NN) problem.

Reference computation (per batch of B=2048 samples):
    h        = tanh(x @ W_h + b_h)                  [B, 512]
    gate     = x @ W_g + b_g                        [B, 1]
    read_key = tanh(h @ W_k + b_k)                  [B, 64]
    kn       = read_key / (||read_key|| + eps)
    Mn       = M / (||M_row|| + eps)                [65536, 64]
    sim      = kn @ Mn.T                            [B, 65536]
    w_read   = softmax(sim, axis=-1)
    r        = w_read @ M                           [B, 64]
    out      = (concat(h, r) @ W_out + b_out)[:, 0] [B]
    returns (out, h[-1], gate[-1], w_read[-1])

Strategy: data-parallel over batch across 8 NeuronCores (256 samples each).
On each core everything is computed in a "transposed" layout (features on
partitions, batch on the free axis) so the streaming pass over the 65536
memory locations needs NO on-chip transposes:

    simT_chunk [128 locs, 256 B] = matmul(lhsT=MnT_chunk [64, 128] bf16,
                                          rhs =knT       [64, 256] bf16)
    w2 = exp(simT)                     (scalar engine, f32 PSUM -> bf16 SBUF)
    rT [65, 256] += matmul(lhsT=[M_chunk | ones] [128, 65] bf16,
                           rhs =w2 [128, 256] bf16)

The ones column folded into the r-matmul accumulates the softmax
denominator Z in row 64 of the same PSUM accumulator.  Cosine similarity
is bounded in [-1, 1] so exp() needs no running-max for stability.

The read-key/knT needed by the streaming loop is computed via a fast bf16
controller pass (~8us) so the memory stream starts early; an fp32
controller pass (for the returned h[-1] and the h @ W_out part of the
output) runs later inside the PE slack of the exp-bound main loop.
"""

import numpy as np

NCORES = 8
B = 2048
IN_DIM = 512
CTRL = 512
N_LOC = 65536
LOC = 64
EPS = 1e-8

BC = B // NCORES          # 256 batch per core
KT = IN_DIM // 128        # 4 input k-tiles
CT = CTRL // 128          # 4 ctrl tiles
NCHUNK = N_LOC // 128     # 512 location chunks
SUPER = 16                # chunks per DMA superblock
NSB = NCHUNK // SUPER     # 32 superblocks
GROUPS = [6, 6, 4]        # exp batching: first two on ScalarE, last on VectorE
DVE_EXP = True            # paired-Schraudolph exp on the Vector engine
LOG2E = 1.4426950408889634
SCH_A = 0.5 * LOG2E * (1 << 23)
SCH_C = 450000.0
SCH_B1 = 127.0 * (1 << 23) - (1 << 21) - SCH_C
SCH_B2 = 127.0 * (1 << 23) + (1 << 21) - SCH_C

_CACHE = {}
DEBUG_TAPS = False


def _build_program():
    import concourse.mybir as mybir
    import concourse.tile as tile
    from concourse import bacc
    from contextlib import ExitStack

    f32 = mybir.dt.float32
    bf16 = mybir.dt.bfloat16
    i32 = mybir.dt.int32
    AF = mybir.ActivationFunctionType
    AOT = mybir.AluOpType

    nc = bacc.Bacc("TRN2", target_bir_lowering=False, debug=False)

    # ---------------- DRAM I/O (per-core shapes; host pre-swizzled) ----------
    xTb = nc.dram_tensor("xTb", [128, KT, BC], bf16, kind="ExternalInput")
    xTl = nc.dram_tensor("xTl", [128, KT, BC], bf16, kind="ExternalInput")
    Whb = nc.dram_tensor("Whb", [128, KT, CTRL], bf16, kind="ExternalInput")
    Whl = nc.dram_tensor("Whl", [128, KT, CTRL], bf16, kind="ExternalInput")
    bh = nc.dram_tensor("bh", [128, CT], f32, kind="ExternalInput")
    Wgb = nc.dram_tensor("Wgb", [128, KT], bf16, kind="ExternalInput")
    Wgl = nc.dram_tensor("Wgl", [128, KT], bf16, kind="ExternalInput")
    bg = nc.dram_tensor("bg", [1, 1], f32, kind="ExternalInput")
    Wkb = nc.dram_tensor("Wkb", [128, KT, LOC], bf16, kind="ExternalInput")
    bk = nc.dram_tensor("bk", [LOC, 1], f32, kind="ExternalInput")
    MnT = nc.dram_tensor("MnT", [LOC, N_LOC], bf16, kind="ExternalInput")
    Mo = nc.dram_tensor("Mo", [128, NCHUNK, LOC + 1], bf16, kind="ExternalInput")
    WoH = nc.dram_tensor("WoH", [128, CT], f32, kind="ExternalInput")
    WoR = nc.dram_tensor("WoR", [LOC, 1], f32, kind="ExternalInput")
    bo = nc.dram_tensor("bo", [1, 1], f32, kind="ExternalInput")

    out_d = nc.dram_tensor("out", [1, BC], f32, kind="ExternalOutput")
    hl_d = nc.dram_tensor("hl", [128, CT], f32, kind="ExternalOutput")
    gt_d = nc.dram_tensor("gt", [1, BC], f32, kind="ExternalOutput")
    wl_d = nc.dram_tensor("wl", [128, NCHUNK], f32, kind="ExternalOutput")
    if DEBUG_TAPS:
        dbg_kn = nc.dram_tensor("dbg_kn", [LOC, BC], f32, kind="ExternalOutput")
        dbg_racc = nc.dram_tensor("dbg_racc", [LOC + 1, BC], f32,
                                  kind="ExternalOutput")

    with tile.TileContext(nc) as tc, ExitStack() as ctx:
        const = ctx.enter_context(tc.tile_pool(name="const", bufs=1))
        mnp = ctx.enter_context(tc.tile_pool(name="mnp", bufs=3))
        mop = ctx.enter_context(tc.tile_pool(name="mop", bufs=3))
        w2p = ctx.enter_context(tc.tile_pool(name="w2p", bufs=3))
        ps_sim = ctx.enter_context(tc.tile_pool(name="ps_sim", bufs=2, space="PSUM"))
        ps_r = ctx.enter_context(tc.tile_pool(name="ps_r", bufs=1, space="PSUM"))
        ps_misc = ctx.enter_context(tc.tile_pool(name="ps_misc", bufs=1, space="PSUM"))

        def tap(dram, ap, shape):
            t = const.tile(shape, f32, tag="tap" + dram.name)
            nc.vector.tensor_copy(out=t[:], in_=ap)
            nc.sync.dma_start(out=dram[:], in_=t[:])

        # ------------- load weights/inputs -------------
        xTb_sb = const.tile([128, KT, BC], bf16)
        nc.sync.dma_start(out=xTb_sb[:], in_=xTb[:])
        xTl_sb = const.tile([128, KT, BC], bf16)
        nc.sync.dma_start(out=xTl_sb[:], in_=xTl[:])
        Whb_sb = const.tile([128, KT, CTRL], bf16)
        nc.sync.dma_start(out=Whb_sb[:], in_=Whb[:])
        Whl_sb = const.tile([128, KT, CTRL], bf16)
        nc.sync.dma_start(out=Whl_sb[:], in_=Whl[:])
        bh_sb = const.tile([128, CT], f32)
        nc.sync.dma_start(out=bh_sb[:], in_=bh[:])
        Wgb_sb = const.tile([128, KT], bf16)
        nc.sync.dma_start(out=Wgb_sb[:], in_=Wgb[:])
        Wgl_sb = const.tile([128, KT], bf16)
        nc.sync.dma_start(out=Wgl_sb[:], in_=Wgl[:])
        bg_sb = const.tile([1, 1], f32)
        nc.sync.dma_start(out=bg_sb[:], in_=bg[:])
        Wkb_sb = const.tile([128, KT, LOC], bf16)
        nc.sync.dma_start(out=Wkb_sb[:], in_=Wkb[:])
        bk_sb = const.tile([LOC, 1], f32)
        nc.sync.dma_start(out=bk_sb[:], in_=bk[:])

        ones_sb = const.tile([128, 128], f32)
        nc.vector.memset(ones_sb[:], 1.0)

        # ------------- controller: h via bf16x2 (hi/lo) => ~fp32 accuracy ---
        hT_sb = const.tile([128, CT, BC], f32)     # for h[-1] export + out_h
        hTb_sb = const.tile([128, CT, BC], bf16)   # for the read-key matmul
        for ct in range(CT):
            ps_h = ps_sim.tile([128, BC], f32, tag="s")
            n = 0
            for k in range(KT):
                cs = slice(ct * 128, (ct + 1) * 128)
                for lhs, rhs in ((Whb_sb[:, k, cs], xTb_sb[:, k, :]),
                                 (Whl_sb[:, k, cs], xTb_sb[:, k, :]),
                                 (Whb_sb[:, k, cs], xTl_sb[:, k, :])):
                    nc.tensor.matmul(ps_h[:], lhs, rhs, start=(n == 0),
                                     stop=(n == 3 * KT - 1))
                    n += 1
            nc.scalar.activation(out=hT_sb[:, ct, :], in_=ps_h[:], func=AF.Tanh,
                                 bias=bh_sb[:, ct:ct + 1], scale=1.0)
            nc.scalar.activation(out=hTb_sb[:, ct, :], in_=ps_h[:], func=AF.Tanh,
                                 bias=bh_sb[:, ct:ct + 1], scale=1.0)

        # gate = x W_g + b_g (bf16x2)
        ps_g = ps_sim.tile([1, BC], f32, tag="s")
        n = 0
        for k in range(KT):
            for lhs, rhs in ((Wgb_sb[:, k:k + 1], xTb_sb[:, k, :]),
                             (Wgl_sb[:, k:k + 1], xTb_sb[:, k, :]),
                             (Wgb_sb[:, k:k + 1], xTl_sb[:, k, :])):
                nc.tensor.matmul(ps_g[:], lhs, rhs, start=(n == 0),
                                 stop=(n == 3 * KT - 1))
                n += 1
        gt_sb = const.tile([1, BC], f32)
        nc.scalar.activation(out=gt_sb[:], in_=ps_g[:], func=AF.Identity,
                             bias=bg_sb[0:1, 0:1], scale=1.0)
        nc.sync.dma_start(out=gt_d[:], in_=gt_sb[:])

        # h[-1]: column BC-1 of hT
        hl_sb = const.tile([128, CT], f32)
        nc.vector.tensor_copy(out=hl_sb[:], in_=hT_sb[:, :, BC - 1])
        nc.sync.dma_start(out=hl_d[:], in_=hl_sb[:])

        ps_rk = ps_sim.tile([LOC, BC], f32, tag="s")
        for k in range(KT):
            nc.tensor.matmul(ps_rk[:], Wkb_sb[:, k, :], hTb_sb[:, k, :],
                             start=(k == 0), stop=(k == KT - 1))
        rkT_sb = const.tile([LOC, BC], bf16)
        nc.scalar.activation(out=rkT_sb[:], in_=ps_rk[:], func=AF.Tanh,
                             bias=bk_sb[:], scale=1.0)

        # kn = rk / ||rk||: sum of squares via ones-matmul, rsqrt via ln/exp
        rksq_sb = const.tile([LOC, BC], f32)
        nc.vector.tensor_mul(rksq_sb[:], rkT_sb[:], rkT_sb[:])
        ps_ss = ps_sim.tile([1, BC], f32, tag="s")
        nc.tensor.matmul(ps_ss[:], ones_sb[0:LOC, 0:1], rksq_sb[:],
                         start=True, stop=True)
        lnss_sb = const.tile([1, BC], f32)
        nc.scalar.activation(out=lnss_sb[:], in_=ps_ss[:], func=AF.Ln)
        invn_sb = const.tile([1, BC], f32)
        nc.scalar.activation(out=invn_sb[:], in_=lnss_sb[:], func=AF.Exp,
                             scale=-0.5)
        ps_bc = ps_sim.tile([LOC, BC], f32, tag="s")
        nc.tensor.matmul(ps_bc[:], ones_sb[0:1, 0:LOC], invn_sb[:],
                         start=True, stop=True)
        knT_sb = const.tile([LOC, BC], bf16)
        nc.vector.tensor_mul(knT_sb[:], rkT_sb[:], ps_bc[:])
        if DEBUG_TAPS:
            tap(dbg_kn, knT_sb[:], [LOC, BC])

        # ------------- streaming pass over the 65536 memory locations -------
        wlast_sb = const.tile([128, NCHUNK], f32)   # unnormalized w of sample BC-1
        ps_rT = ps_r.tile([LOC + 1, BC], f32)       # rows 0..63: r^T; row 64: Z

        def emit_r(mo_tile, w2_tile, base, g0, gsz):
            for j in range(g0, g0 + gsz):
                c = base + j
                nc.tensor.matmul(ps_rT[:], mo_tile[:, j, :], w2_tile[:, j, :],
                                 start=(c == 0), stop=(c == NCHUNK - 1))

        G0, G1, G2 = GROUPS
        pending = None   # DVE-exp'd group of the previous superblock
        for sb in range(NSB):
            c0 = sb * SUPER
            mn_t = mnp.tile([LOC, SUPER, 128], bf16)
            nc.sync.dma_start(
                out=mn_t[:],
                in_=MnT[:, c0 * 128:(c0 + SUPER) * 128].rearrange(
                    "f (s c) -> f s c", s=SUPER),
            )
            mo_t = mop.tile([128, SUPER, LOC + 1], bf16)
            nc.sync.dma_start(out=mo_t[:], in_=Mo[:, c0:c0 + SUPER, :])
            w2_t = w2p.tile([128, SUPER, BC], bf16)

            # sims of group 0 and 1
            ps_s0 = ps_sim.tile([128, G0, BC], f32, tag="s")
            for j in range(G0):
                nc.tensor.matmul(ps_s0[:, j, :], mn_t[:, j, :], knT_sb[:],
                                 start=True, stop=True)
            nc.scalar.activation(out=w2_t[:, 0:G0, :], in_=ps_s0[:], func=AF.Exp)
            ps_s1 = ps_sim.tile([128, G1, BC], f32, tag="s")
            for j in range(G1):
                nc.tensor.matmul(ps_s1[:, j, :], mn_t[:, G0 + j, :], knT_sb[:],
                                 start=True, stop=True)
            nc.scalar.activation(out=w2_t[:, G0:G0 + G1, :], in_=ps_s1[:],
                                 func=AF.Exp)
            # r-matmuls of the previous superblock's DVE group (full sb slack)
            if pending is not None:
                emit_r(*pending)
                pending = None
            emit_r(mo_t, w2_t, c0, 0, G0)
            # sims of group 2 + exp on the Vector engine
            ps_s2 = ps_sim.tile([128, G2, BC], f32, tag="s")
            for j in range(G2):
                nc.tensor.matmul(ps_s2[:, j, :], mn_t[:, G0 + G1 + j, :],
                                 knT_sb[:], start=True, stop=True)
            if DVE_EXP:
                # exp on VectorE: product of two phase-shifted Schraudolph
                # approximations (bitcast(int32(x*A+B)) ~ 2^(x*log2e/2))
                y1_t = w2p.tile([128, G2, BC], i32, tag="y1")
                nc.vector.tensor_scalar(out=y1_t[:], in0=ps_s2[:],
                                        scalar1=SCH_A, scalar2=SCH_B1,
                                        op0=AOT.mult, op1=AOT.add)
                y2_t = w2p.tile([128, G2, BC], i32, tag="y2")
                nc.vector.tensor_scalar(out=y2_t[:], in0=ps_s2[:],
                                        scalar1=SCH_A, scalar2=SCH_B2,
                                        op0=AOT.mult, op1=AOT.add)
                nc.vector.tensor_mul(w2_t[:, G0 + G1:, :],
                                     y1_t[:].bitcast(f32), y2_t[:].bitcast(f32))
                pending = (mo_t, w2_t, c0, G0 + G1, G2)
            else:
                nc.scalar.activation(out=w2_t[:, G0 + G1:, :], in_=ps_s2[:],
                                     func=AF.Exp)
            emit_r(mo_t, w2_t, c0, G0, G1)
            if not DVE_EXP:
                emit_r(mo_t, w2_t, c0, G0 + G1, G2)
            nc.gpsimd.tensor_copy(out=wlast_sb[:, c0:c0 + SUPER],
                                  in_=w2_t[:, :, BC - 1])
        if pending is not None:
            emit_r(*pending)
            pending = None
        if DEBUG_TAPS:
            tap(dbg_racc, ps_rT[:], [LOC + 1, BC])

        # ------------- epilogue weights ---------------------------------------
        WoH_sb = const.tile([128, CT], f32)
        nc.sync.dma_start(out=WoH_sb[:], in_=WoH[:])
        WoR_sb = const.tile([LOC, 1], f32)
        nc.sync.dma_start(out=WoR_sb[:], in_=WoR[:])
        bo_sb = const.tile([1, 1], f32)
        nc.sync.dma_start(out=bo_sb[:], in_=bo[:])

        # ------------- epilogue ---------------------------------------------
        # 1/Z on partition 64 (where Z landed), then PE-broadcasts
        zw_sb = const.tile([128, BC], f32)
        nc.vector.reciprocal(out=zw_sb[64:65, :], in_=ps_rT[LOC:LOC + 1, :])

        racc_sb = const.tile([LOC, BC], f32)
        nc.vector.tensor_copy(out=racc_sb[:], in_=ps_rT[0:LOC, :])

        # broadcast 1/Z from partition 64 to partition 0 (row 0 of [64, BC])
        ps_zb = ps_misc.tile([LOC, BC], f32, tag="m")
        nc.tensor.matmul(ps_zb[:], ones_sb[64:65, 0:LOC], zw_sb[64:65, :],
                         start=True, stop=True)
        zb_sb = const.tile([LOC, BC], f32)
        nc.vector.tensor_copy(out=zb_sb[:], in_=ps_zb[:])

        # out_r = (rT^T WoR) / Z as [1, 256]
        ps_or = ps_misc.tile([1, BC], f32, tag="m")
        nc.tensor.matmul(ps_or[:], WoR_sb[:], racc_sb[:], start=True, stop=True)
        t1_sb = const.tile([1, BC], f32)
        nc.vector.tensor_mul(t1_sb[:], zb_sb[0:1, :], ps_or[:])

        # out_h = hT^T WoH as [1, 256]; final = out_h + out_r + bo
        ps_oh = ps_misc.tile([1, BC], f32, tag="m")
        for k in range(CT):
            nc.tensor.matmul(ps_oh[:], WoH_sb[:, k:k + 1], hT_sb[:, k, :],
                             start=(k == 0), stop=(k == CT - 1))
        t2_sb = const.tile([1, BC], f32)
        nc.vector.tensor_add(t2_sb[:], t1_sb[:], ps_oh[:])
        outv_sb = const.tile([1, BC], f32)
        nc.vector.tensor_scalar_add(outv_sb[:], t2_sb[:], bo_sb[0:1, 0:1])
        nc.sync.dma_start(out=out_d[:], in_=outv_sb[:])

        # w_read[-1] = wlast * (1/Z[BC-1]) broadcast to all 128 partitions
        ps_zl = ps_misc.tile([128, 1], f32, tag="m")
        nc.tensor.matmul(ps_zl[:], ones_sb[64:65, :], zw_sb[64:65, BC - 1:BC],
                         start=True, stop=True)
        zl_sb = const.tile([128, 1], f32)
        nc.vector.tensor_copy(out=zl_sb[:], in_=ps_zl[:])
        wlf_sb = const.tile([128, NCHUNK], f32)
        nc.vector.tensor_scalar_mul(wlf_sb[:], wlast_sb[:], zl_sb[:, 0:1])
        nc.sync.dma_start(out=wl_d[:], in_=wlf_sb[:])

    nc.compile()
    return nc


def _get_program():
    if "nc" not in _CACHE:
        _CACHE["nc"] = _build_program()
    return _CACHE["nc"]


def _prep_in_maps(inputs):
    return _prep(**{k: np.asarray(v) for k, v in inputs.items()})


def _prep(x, W_h, b_h, W_g, b_g, W_k, b_k, M, W_out, b_out):
    import ml_dtypes
    bf = ml_dtypes.bfloat16

    x = np.ascontiguousarray(np.asarray(x, dtype=np.float32))
    W_h = np.asarray(W_h, dtype=np.float32)
    b_h = np.asarray(b_h, dtype=np.float32)
    W_g = np.asarray(W_g, dtype=np.float32)
    b_g = np.asarray(b_g, dtype=np.float32)
    W_k = np.asarray(W_k, dtype=np.float32)
    b_k = np.asarray(b_k, dtype=np.float32)
    M = np.ascontiguousarray(np.asarray(M, dtype=np.float32))
    W_out = np.asarray(W_out, dtype=np.float32)
    b_out = np.asarray(b_out, dtype=np.float32)

    # ---- host-side layout prep (weight swizzles for SBUF-friendly DMA) ----
    norms = np.linalg.norm(M, axis=1, keepdims=True)
    MnT = np.ascontiguousarray((M / (norms + EPS)).T).astype(bf)  # [64, 65536]
    Mo = np.concatenate([M, np.ones((N_LOC, 1), np.float32)], axis=1)
    Mo = np.ascontiguousarray(
        Mo.reshape(NCHUNK, 128, LOC + 1).transpose(1, 0, 2)).astype(bf)

    Wh_p = np.ascontiguousarray(W_h.reshape(KT, 128, CTRL).transpose(1, 0, 2))
    bh_p = np.ascontiguousarray(b_h.reshape(CT, 128).T)
    Wg_p = np.ascontiguousarray(W_g[:, 0].reshape(KT, 128).T)
    bg_p = b_g.reshape(1, 1)
    Wk_p = np.ascontiguousarray(W_k.reshape(KT, 128, LOC).transpose(1, 0, 2))
    bk_p = b_k.reshape(LOC, 1)
    WoH_p = np.ascontiguousarray(W_out[:CTRL, 0].reshape(CT, 128).T)
    WoR_p = np.ascontiguousarray(W_out[CTRL:, 0:1])
    bo_p = b_out.reshape(1, 1)

    def hilo(a):
        hi = a.astype(bf)
        lo = (a - hi.astype(np.float32)).astype(bf)
        return hi, lo

    Whb_p, Whl_p = hilo(Wh_p)
    Wgb_p, Wgl_p = hilo(Wg_p)
    shared = dict(Whb=Whb_p, Whl=Whl_p, bh=bh_p, Wgb=Wgb_p, Wgl=Wgl_p, bg=bg_p,
                  Wkb=Wk_p.astype(bf), bk=bk_p, MnT=MnT, Mo=Mo,
                  WoH=WoH_p, WoR=WoR_p, bo=bo_p)
    in_maps = []
    for c in range(NCORES):
        xc = x[c * BC:(c + 1) * BC]                              # [256, 512]
        xT_p = np.ascontiguousarray(
            xc.T.reshape(KT, 128, BC).transpose(1, 0, 2))        # [128, 4, 256]
        xb, xl = hilo(xT_p)
        in_maps.append(dict(shared, xTb=xb, xTl=xl))
    return in_maps


def kernel(x, W_h, b_h, W_g, b_g, W_k, b_k, M, W_out, b_out):
    from concourse.bass_utils import run_bass_kernel_spmd

    in_maps = _prep(x, W_h, b_h, W_g, b_g, W_k, b_k, M, W_out, b_out)
    nc = _get_program()
    res = run_bass_kernel_spmd(nc, in_maps, core_ids=list(range(NCORES))).results

    output = np.concatenate([res[c]["out"][0] for c in range(NCORES)])
    h_last = np.ascontiguousarray(res[NCORES - 1]["hl"].T).reshape(CTRL)
    gate_last = res[NCORES - 1]["gt"][0, BC - 1:BC].copy()
    w_read_last = np.ascontiguousarray(res[NCORES - 1]["wl"].T).reshape(N_LOC)
    return (output.astype(np.float32), h_last.astype(np.float32),
            gate_last.astype(np.float32), w_read_last.astype(np.float32))


# revision 16
# speedup vs baseline: 1.3765x; 1.1315x over previous
"""Trainium2 Bass kernel for the MANN (memory-augmented NN) problem.

Reference computation (per batch of B=2048 samples):
    h        = tanh(x @ W_h + b_h)                  [B, 512]
    gate     = x @ W_g + b_g                        [B, 1]
    read_key = tanh(h @ W_k + b_k)                  [B, 64]
    kn       = read_key / (||read_key|| + eps)
    Mn       = M / (||M_row|| + eps)                [65536, 64]
    sim      = kn @ Mn.T                            [B, 65536]
    w_read   = softmax(sim, axis=-1)
    r        = w_read @ M                           [B, 64]
    out      = (concat(h, r) @ W_out + b_out)[:, 0] [B]
    returns (out, h[-1], gate[-1], w_read[-1])

Strategy: data-parallel over batch across 8 NeuronCores (256 samples each).
On each core everything is computed in a "transposed" layout (features on
partitions, batch on the free axis) so the streaming pass over the 65536
memory locations needs NO on-chip transposes:

    simT_chunk [128 locs, 256 B] = matmul(lhsT=MnT_chunk [64, 128] bf16,
                                          rhs =knT       [64, 256] bf16)
    w2 = exp(simT)                     (scalar engine, f32 PSUM -> bf16 SBUF)
    rT [65, 256] += matmul(lhsT=[M_chunk | ones] [128, 65] bf16,
                           rhs =w2 [128, 256] bf16)

The ones column folded into the r-matmul accumulates the softmax
denominator Z in row 64 of the same PSUM accumulator.  Cosine similarity
is bounded in [-1, 1] so exp() needs no running-max for stability.

The read-key/knT needed by the streaming loop is computed via a fast bf16
controller pass (~8us) so the memory stream starts early; an fp32
controller pass (for the returned h[-1] and the h @ W_out part of the
output) runs later inside the PE slack of the exp-bound main loop.
"""

import numpy as np

NCORES = 8
B = 2048
IN_DIM = 512
CTRL = 512
N_LOC = 65536
LOC = 64
EPS = 1e-8

BC = B // NCORES          # 256 batch per core
KT = IN_DIM // 128        # 4 input k-tiles
CT = CTRL // 128          # 4 ctrl tiles
NCHUNK = N_LOC // 128     # 512 location chunks
SUPER = 16                # chunks per DMA superblock
NSB = NCHUNK // SUPER     # 32 superblocks
GROUPS = [4, 4, 4, 4]     # exp batching: group 0 on VectorE, rest on ScalarE
DVE_EXP = True            # paired-Schraudolph exp on the Vector engine
LOG2E = 1.4426950408889634
SCH_A = 0.5 * LOG2E * (1 << 23)
SCH_C = 450000.0
SCH_B1 = 127.0 * (1 << 23) - (1 << 21) - SCH_C
SCH_B2 = 127.0 * (1 << 23) + (1 << 21) - SCH_C

_CACHE = {}
DEBUG_TAPS = False


def _build_program():
    import concourse.mybir as mybir
    import concourse.tile as tile
    from concourse import bacc
    from contextlib import ExitStack

    f32 = mybir.dt.float32
    bf16 = mybir.dt.bfloat16
    i32 = mybir.dt.int32
    AF = mybir.ActivationFunctionType
    AOT = mybir.AluOpType

    nc = bacc.Bacc("TRN2", target_bir_lowering=False, debug=False)

    # ---------------- DRAM I/O (per-core shapes; host pre-swizzled) ----------
    xTb = nc.dram_tensor("xTb", [128, KT, BC], bf16, kind="ExternalInput")
    xTl = nc.dram_tensor("xTl", [128, KT, BC], bf16, kind="ExternalInput")
    Whb = nc.dram_tensor("Whb", [128, KT, CTRL], bf16, kind="ExternalInput")
    Whl = nc.dram_tensor("Whl", [128, KT, CTRL], bf16, kind="ExternalInput")
    bh = nc.dram_tensor("bh", [128, CT], f32, kind="ExternalInput")
    Wgb = nc.dram_tensor("Wgb", [128, KT], bf16, kind="ExternalInput")
    Wgl = nc.dram_tensor("Wgl", [128, KT], bf16, kind="ExternalInput")
    bg = nc.dram_tensor("bg", [1, 1], f32, kind="ExternalInput")
    Wkb = nc.dram_tensor("Wkb", [128, KT, LOC], bf16, kind="ExternalInput")
    bk = nc.dram_tensor("bk", [LOC, 1], f32, kind="ExternalInput")
    MnT = nc.dram_tensor("MnT", [LOC, N_LOC], bf16, kind="ExternalInput")
    Mo = nc.dram_tensor("Mo", [128, NCHUNK, LOC + 1], bf16, kind="ExternalInput")
    WoH = nc.dram_tensor("WoH", [128, CT], f32, kind="ExternalInput")
    WoR = nc.dram_tensor("WoR", [LOC, 1], f32, kind="ExternalInput")
    bo = nc.dram_tensor("bo", [1, 1], f32, kind="ExternalInput")

    out_d = nc.dram_tensor("out", [1, BC], f32, kind="ExternalOutput")
    hl_d = nc.dram_tensor("hl", [128, CT], f32, kind="ExternalOutput")
    gt_d = nc.dram_tensor("gt", [1, BC], f32, kind="ExternalOutput")
    wl_d = nc.dram_tensor("wl", [128, NCHUNK], f32, kind="ExternalOutput")
    if DEBUG_TAPS:
        dbg_kn = nc.dram_tensor("dbg_kn", [LOC, BC], f32, kind="ExternalOutput")
        dbg_racc = nc.dram_tensor("dbg_racc", [LOC + 1, BC], f32,
                                  kind="ExternalOutput")

    with tile.TileContext(nc) as tc, ExitStack() as ctx:
        const = ctx.enter_context(tc.tile_pool(name="const", bufs=1))
        mnp = ctx.enter_context(tc.tile_pool(name="mnp", bufs=3))
        mop = ctx.enter_context(tc.tile_pool(name="mop", bufs=3))
        w2p = ctx.enter_context(tc.tile_pool(name="w2p", bufs=3))
        ps_sim = ctx.enter_context(tc.tile_pool(name="ps_sim", bufs=3, space="PSUM"))
        ps_r = ctx.enter_context(tc.tile_pool(name="ps_r", bufs=1, space="PSUM"))
        ps_misc = ctx.enter_context(tc.tile_pool(name="ps_misc", bufs=1, space="PSUM"))

        def tap(dram, ap, shape):
            t = const.tile(shape, f32, tag="tap" + dram.name)
            nc.vector.tensor_copy(out=t[:], in_=ap)
            nc.sync.dma_start(out=dram[:], in_=t[:])

        # ------------- load weights/inputs -------------
        xTb_sb = const.tile([128, KT, BC], bf16)
        nc.sync.dma_start(out=xTb_sb[:], in_=xTb[:])
        xTl_sb = const.tile([128, KT, BC], bf16)
        nc.sync.dma_start(out=xTl_sb[:], in_=xTl[:])
        Whb_sb = const.tile([128, KT, CTRL], bf16)
        nc.sync.dma_start(out=Whb_sb[:], in_=Whb[:])
        Whl_sb = const.tile([128, KT, CTRL], bf16)
        nc.sync.dma_start(out=Whl_sb[:], in_=Whl[:])
        bh_sb = const.tile([128, CT], f32)
        nc.sync.dma_start(out=bh_sb[:], in_=bh[:])
        Wgb_sb = const.tile([128, KT], bf16)
        nc.sync.dma_start(out=Wgb_sb[:], in_=Wgb[:])
        Wgl_sb = const.tile([128, KT], bf16)
        nc.sync.dma_start(out=Wgl_sb[:], in_=Wgl[:])
        bg_sb = const.tile([1, 1], f32)
        nc.sync.dma_start(out=bg_sb[:], in_=bg[:])
        Wkb_sb = const.tile([128, KT, LOC], bf16)
        nc.sync.dma_start(out=Wkb_sb[:], in_=Wkb[:])
        bk_sb = const.tile([LOC, 1], f32)
        nc.sync.dma_start(out=bk_sb[:], in_=bk[:])

        ones_sb = const.tile([128, 128], f32)
        nc.vector.memset(ones_sb[:], 1.0)

        # ------------- controller: h via bf16x2 (hi/lo) => ~fp32 accuracy ---
        hT_sb = const.tile([128, CT, BC], f32)     # for h[-1] export + out_h
        hTb_sb = const.tile([128, CT, BC], bf16)   # for the read-key matmul
        for ct in range(CT):
            ps_h = ps_sim.tile([128, BC], f32, tag="s")
            n = 0
            for k in range(KT):
                cs = slice(ct * 128, (ct + 1) * 128)
                for lhs, rhs in ((Whb_sb[:, k, cs], xTb_sb[:, k, :]),
                                 (Whl_sb[:, k, cs], xTb_sb[:, k, :]),
                                 (Whb_sb[:, k, cs], xTl_sb[:, k, :])):
                    nc.tensor.matmul(ps_h[:], lhs, rhs, start=(n == 0),
                                     stop=(n == 3 * KT - 1))
                    n += 1
            nc.scalar.activation(out=hT_sb[:, ct, :], in_=ps_h[:], func=AF.Tanh,
                                 bias=bh_sb[:, ct:ct + 1], scale=1.0)
            nc.scalar.activation(out=hTb_sb[:, ct, :], in_=ps_h[:], func=AF.Tanh,
                                 bias=bh_sb[:, ct:ct + 1], scale=1.0)

        # gate = x W_g + b_g (bf16x2)
        ps_g = ps_sim.tile([1, BC], f32, tag="s")
        n = 0
        for k in range(KT):
            for lhs, rhs in ((Wgb_sb[:, k:k + 1], xTb_sb[:, k, :]),
                             (Wgl_sb[:, k:k + 1], xTb_sb[:, k, :]),
                             (Wgb_sb[:, k:k + 1], xTl_sb[:, k, :])):
                nc.tensor.matmul(ps_g[:], lhs, rhs, start=(n == 0),
                                 stop=(n == 3 * KT - 1))
                n += 1
        gt_sb = const.tile([1, BC], f32)
        nc.scalar.activation(out=gt_sb[:], in_=ps_g[:], func=AF.Identity,
                             bias=bg_sb[0:1, 0:1], scale=1.0)
        nc.sync.dma_start(out=gt_d[:], in_=gt_sb[:])

        # h[-1]: column BC-1 of hT
        hl_sb = const.tile([128, CT], f32)
        nc.vector.tensor_copy(out=hl_sb[:], in_=hT_sb[:, :, BC - 1])
        nc.sync.dma_start(out=hl_d[:], in_=hl_sb[:])

        ps_rk = ps_sim.tile([LOC, BC], f32, tag="s")
        for k in range(KT):
            nc.tensor.matmul(ps_rk[:], Wkb_sb[:, k, :], hTb_sb[:, k, :],
                             start=(k == 0), stop=(k == KT - 1))
        rkT_sb = const.tile([LOC, BC], bf16)
        nc.scalar.activation(out=rkT_sb[:], in_=ps_rk[:], func=AF.Tanh,
                             bias=bk_sb[:], scale=1.0)

        # kn = rk / ||rk||: sum of squares via ones-matmul, rsqrt via ln/exp
        rksq_sb = const.tile([LOC, BC], f32)
        nc.vector.tensor_mul(rksq_sb[:], rkT_sb[:], rkT_sb[:])
        ps_ss = ps_sim.tile([1, BC], f32, tag="s")
        nc.tensor.matmul(ps_ss[:], ones_sb[0:LOC, 0:1], rksq_sb[:],
                         start=True, stop=True)
        lnss_sb = const.tile([1, BC], f32)
        nc.scalar.activation(out=lnss_sb[:], in_=ps_ss[:], func=AF.Ln)
        invn_sb = const.tile([1, BC], f32)
        nc.scalar.activation(out=invn_sb[:], in_=lnss_sb[:], func=AF.Exp,
                             scale=-0.5)
        ps_bc = ps_sim.tile([LOC, BC], f32, tag="s")
        nc.tensor.matmul(ps_bc[:], ones_sb[0:1, 0:LOC], invn_sb[:],
                         start=True, stop=True)
        knT_sb = const.tile([LOC, BC], bf16)
        nc.vector.tensor_mul(knT_sb[:], rkT_sb[:], ps_bc[:])
        if DEBUG_TAPS:
            tap(dbg_kn, knT_sb[:], [LOC, BC])

        # ------------- streaming pass over the 65536 memory locations -------
        wlast_sb = const.tile([128, NCHUNK], f32)   # unnormalized w of sample BC-1
        ps_rT = ps_r.tile([LOC + 1, BC], f32)       # rows 0..63: r^T; row 64: Z

        r_count = [0]

        def emit_r(mo_tile, w2_tile, base, g0, gsz):
            # start/stop must follow EMISSION (=execution) order, not chunk
            # index: a late start=True would wipe prior accumulation.
            for j in range(g0, g0 + gsz):
                nc.tensor.matmul(ps_rT[:], mo_tile[:, j, :], w2_tile[:, j, :],
                                 start=(r_count[0] == 0),
                                 stop=(r_count[0] == NCHUNK - 1))
                r_count[0] += 1

        G = GROUPS[0]
        NG = SUPER // G                        # groups per superblock
        pending = None   # DVE-exp'd group of the previous superblock
        for sb in range(NSB):
            c0 = sb * SUPER
            mn_t = mnp.tile([LOC, SUPER, 128], bf16)
            nc.sync.dma_start(
                out=mn_t[:],
                in_=MnT[:, c0 * 128:(c0 + SUPER) * 128].rearrange(
                    "f (s c) -> f s c", s=SUPER),
            )
            mo_t = mop.tile([128, SUPER, LOC + 1], bf16)
            nc.sync.dma_start(out=mo_t[:], in_=Mo[:, c0:c0 + SUPER, :])
            w2_t = w2p.tile([128, SUPER, BC], bf16)

            # group 0: sims then exp on the Vector engine (max slack: its
            # r-matmuls are deferred to the next superblock)
            ps_s0 = ps_sim.tile([128, G, BC], f32, tag="s")
            for j in range(G):
                nc.tensor.matmul(ps_s0[:, j, :], mn_t[:, j, :], knT_sb[:],
                                 start=True, stop=True)
            if DVE_EXP:
                # exp via product of two phase-shifted Schraudolph
                # approximations: bitcast(int32(x*A+B)) ~ 2^(x*log2e/2)
                y1_t = w2p.tile([128, G, BC], i32, tag="y1")
                nc.vector.tensor_scalar(out=y1_t[:], in0=ps_s0[:],
                                        scalar1=SCH_A, scalar2=SCH_B1,
                                        op0=AOT.mult, op1=AOT.add)
                y2_t = w2p.tile([128, G, BC], i32, tag="y2")
                nc.vector.tensor_scalar(out=y2_t[:], in0=ps_s0[:],
                                        scalar1=SCH_A, scalar2=SCH_B2,
                                        op0=AOT.mult, op1=AOT.add)
                nc.vector.tensor_mul(w2_t[:, 0:G, :],
                                     y1_t[:].bitcast(f32), y2_t[:].bitcast(f32))
            else:
                nc.scalar.activation(out=w2_t[:, 0:G, :], in_=ps_s0[:],
                                     func=AF.Exp)
            # groups 1..NG-1: sims + ACT exp, r-matmuls interleaved
            for gi in range(1, NG):
                g0 = gi * G
                ps_s = ps_sim.tile([128, G, BC], f32, tag="s")
                for j in range(G):
                    nc.tensor.matmul(ps_s[:, j, :], mn_t[:, g0 + j, :],
                                     knT_sb[:], start=True, stop=True)
                nc.scalar.activation(out=w2_t[:, g0:g0 + G, :], in_=ps_s[:],
                                     func=AF.Exp)
                if gi == 1:
                    if pending is not None:
                        emit_r(*pending)
                        pending = None
                    if not DVE_EXP:
                        emit_r(mo_t, w2_t, c0, 0, G)
                else:
                    emit_r(mo_t, w2_t, c0, (gi - 1) * G, G)
            emit_r(mo_t, w2_t, c0, (NG - 1) * G, G)
            if DVE_EXP:
                pending = (mo_t, w2_t, c0, 0, G)
            nc.gpsimd.tensor_copy(out=wlast_sb[:, c0:c0 + SUPER],
                                  in_=w2_t[:, :, BC - 1])
        if pending is not None:
            emit_r(*pending)
            pending = None
        if DEBUG_TAPS:
            tap(dbg_racc, ps_rT[:], [LOC + 1, BC])

        # ------------- epilogue weights ---------------------------------------
        WoH_sb = const.tile([128, CT], f32)
        nc.sync.dma_start(out=WoH_sb[:], in_=WoH[:])
        WoR_sb = const.tile([LOC, 1], f32)
        nc.sync.dma_start(out=WoR_sb[:], in_=WoR[:])
        bo_sb = const.tile([1, 1], f32)
        nc.sync.dma_start(out=bo_sb[:], in_=bo[:])

        # ------------- epilogue ---------------------------------------------
        # 1/Z on partition 64 (where Z landed), then PE-broadcasts
        zw_sb = const.tile([128, BC], f32)
        nc.vector.reciprocal(out=zw_sb[64:65, :], in_=ps_rT[LOC:LOC + 1, :])

        racc_sb = const.tile([LOC, BC], f32)
        nc.vector.tensor_copy(out=racc_sb[:], in_=ps_rT[0:LOC, :])

        # broadcast 1/Z from partition 64 to partition 0 (row 0 of [64, BC])
        ps_zb = ps_misc.tile([LOC, BC], f32, tag="m")
        nc.tensor.matmul(ps_zb[:], ones_sb[64:65, 0:LOC], zw_sb[64:65, :],
                         start=True, stop=True)
        zb_sb = const.tile([LOC, BC], f32)
        nc.vector.tensor_copy(out=zb_sb[:], in_=ps_zb[:])

        # out_r = (rT^T WoR) / Z as [1, 256]
        ps_or = ps_misc.tile([1, BC], f32, tag="m")
        nc.tensor.matmul(ps_or[:], WoR_sb[:], racc_sb[:], start=True, stop=True)
        t1_sb = const.tile([1, BC], f32)
        nc.vector.tensor_mul(t1_sb[:], zb_sb[0:1, :], ps_or[:])

        # out_h = hT^T WoH as [1, 256]; final = out_h + out_r + bo
        ps_oh = ps_misc.tile([1, BC], f32, tag="m")
        for k in range(CT):
            nc.tensor.matmul(ps_oh[:], WoH_sb[:, k:k + 1], hT_sb[:, k, :],
                             start=(k == 0), stop=(k == CT - 1))
        t2_sb = const.tile([1, BC], f32)
        nc.vector.tensor_add(t2_sb[:], t1_sb[:], ps_oh[:])
        outv_sb = const.tile([1, BC], f32)
        nc.vector.tensor_scalar_add(outv_sb[:], t2_sb[:], bo_sb[0:1, 0:1])
        nc.sync.dma_start(out=out_d[:], in_=outv_sb[:])

        # w_read[-1] = wlast * (1/Z[BC-1]) broadcast to all 128 partitions
        ps_zl = ps_misc.tile([128, 1], f32, tag="m")
        nc.tensor.matmul(ps_zl[:], ones_sb[64:65, :], zw_sb[64:65, BC - 1:BC],
                         start=True, stop=True)
        zl_sb = const.tile([128, 1], f32)
        nc.vector.tensor_copy(out=zl_sb[:], in_=ps_zl[:])
        wlf_sb = const.tile([128, NCHUNK], f32)
        nc.vector.tensor_scalar_mul(wlf_sb[:], wlast_sb[:], zl_sb[:, 0:1])
        nc.sync.dma_start(out=wl_d[:], in_=wlf_sb[:])

    nc.compile()
    return nc


def _get_program():
    if "nc" not in _CACHE:
        _CACHE["nc"] = _build_program()
    return _CACHE["nc"]


def _prep_in_maps(inputs):
    return _prep(**{k: np.asarray(v) for k, v in inputs.items()})


def _prep(x, W_h, b_h, W_g, b_g, W_k, b_k, M, W_out, b_out):
    import ml_dtypes
    bf = ml_dtypes.bfloat16

    x = np.ascontiguousarray(np.asarray(x, dtype=np.float32))
    W_h = np.asarray(W_h, dtype=np.float32)
    b_h = np.asarray(b_h, dtype=np.float32)
    W_g = np.asarray(W_g, dtype=np.float32)
    b_g = np.asarray(b_g, dtype=np.float32)
    W_k = np.asarray(W_k, dtype=np.float32)
    b_k = np.asarray(b_k, dtype=np.float32)
    M = np.ascontiguousarray(np.asarray(M, dtype=np.float32))
    W_out = np.asarray(W_out, dtype=np.float32)
    b_out = np.asarray(b_out, dtype=np.float32)

    # ---- host-side layout prep (weight swizzles for SBUF-friendly DMA) ----
    norms = np.linalg.norm(M, axis=1, keepdims=True)
    MnT = np.ascontiguousarray((M / (norms + EPS)).T).astype(bf)  # [64, 65536]
    Mo = np.concatenate([M, np.ones((N_LOC, 1), np.float32)], axis=1)
    Mo = np.ascontiguousarray(
        Mo.reshape(NCHUNK, 128, LOC + 1).transpose(1, 0, 2)).astype(bf)

    Wh_p = np.ascontiguousarray(W_h.reshape(KT, 128, CTRL).transpose(1, 0, 2))
    bh_p = np.ascontiguousarray(b_h.reshape(CT, 128).T)
    Wg_p = np.ascontiguousarray(W_g[:, 0].reshape(KT, 128).T)
    bg_p = b_g.reshape(1, 1)
    Wk_p = np.ascontiguousarray(W_k.reshape(KT, 128, LOC).transpose(1, 0, 2))
    bk_p = b_k.reshape(LOC, 1)
    WoH_p = np.ascontiguousarray(W_out[:CTRL, 0].reshape(CT, 128).T)
    WoR_p = np.ascontiguousarray(W_out[CTRL:, 0:1])
    bo_p = b_out.reshape(1, 1)

    def hilo(a):
        hi = a.astype(bf)
        lo = (a - hi.astype(np.float32)).astype(bf)
        return hi, lo

    Whb_p, Whl_p = hilo(Wh_p)
    Wgb_p, Wgl_p = hilo(Wg_p)
    shared = dict(Whb=Whb_p, Whl=Whl_p, bh=bh_p, Wgb=Wgb_p, Wgl=Wgl_p, bg=bg_p,
                  Wkb=Wk_p.astype(bf), bk=bk_p, MnT=MnT, Mo=Mo,
                  WoH=WoH_p, WoR=WoR_p, bo=bo_p)
    in_maps = []
    for c in range(NCORES):
        xc = x[c * BC:(c + 1) * BC]                              # [256, 512]
        xT_p = np.ascontiguousarray(
            xc.T.reshape(KT, 128, BC).transpose(1, 0, 2))        # [128, 4, 256]
        xb, xl = hilo(xT_p)
        in_maps.append(dict(shared, xTb=xb, xTl=xl))
    return in_maps


def kernel(x, W_h, b_h, W_g, b_g, W_k, b_k, M, W_out, b_out):
    from concourse.bass_utils import run_bass_kernel_spmd

    in_maps = _prep(x, W_h, b_h, W_g, b_g, W_k, b_k, M, W_out, b_out)
    nc = _get_program()
    res = run_bass_kernel_spmd(nc, in_maps, core_ids=list(range(NCORES))).results

    output = np.concatenate([res[c]["out"][0] for c in range(NCORES)])
    h_last = np.ascontiguousarray(res[NCORES - 1]["hl"].T).reshape(CTRL)
    gate_last = res[NCORES - 1]["gt"][0, BC - 1:BC].copy()
    w_read_last = np.ascontiguousarray(res[NCORES - 1]["wl"].T).reshape(N_LOC)
    return (output.astype(np.float32), h_last.astype(np.float32),
            gate_last.astype(np.float32), w_read_last.astype(np.float32))


# revision 19
# speedup vs baseline: 1.4091x; 1.0237x over previous
"""Trainium2 Bass kernel for the MANN (memory-augmented NN) problem.

Reference computation (per batch of B=2048 samples):
    h        = tanh(x @ W_h + b_h)                  [B, 512]
    gate     = x @ W_g + b_g                        [B, 1]
    read_key = tanh(h @ W_k + b_k)                  [B, 64]
    kn       = read_key / (||read_key|| + eps)
    Mn       = M / (||M_row|| + eps)                [65536, 64]
    sim      = kn @ Mn.T                            [B, 65536]
    w_read   = softmax(sim, axis=-1)
    r        = w_read @ M                           [B, 64]
    out      = (concat(h, r) @ W_out + b_out)[:, 0] [B]
    returns (out, h[-1], gate[-1], w_read[-1])

Strategy: data-parallel over batch across 8 NeuronCores (256 samples each).
On each core everything is computed in a "transposed" layout (features on
partitions, batch on the free axis) so the streaming pass over the 65536
memory locations needs NO on-chip transposes:

    simT_chunk [128 locs, 256 B] = matmul(lhsT=MnT_chunk [64, 128] bf16,
                                          rhs =knT       [64, 256] bf16)
    w2 = exp(simT)                     (scalar engine, f32 PSUM -> bf16 SBUF)
    rT [65, 256] += matmul(lhsT=[M_chunk | ones] [128, 65] bf16,
                           rhs =w2 [128, 256] bf16)

The ones column folded into the r-matmul accumulates the softmax
denominator Z in row 64 of the same PSUM accumulator.  Cosine similarity
is bounded in [-1, 1] so exp() needs no running-max for stability.

The read-key/knT needed by the streaming loop is computed via a fast bf16
controller pass (~8us) so the memory stream starts early; an fp32
controller pass (for the returned h[-1] and the h @ W_out part of the
output) runs later inside the PE slack of the exp-bound main loop.
"""

import numpy as np

NCORES = 8
B = 2048
IN_DIM = 512
CTRL = 512
N_LOC = 65536
LOC = 64
EPS = 1e-8

BC = B // NCORES          # 256 batch per core
KT = IN_DIM // 128        # 4 input k-tiles
CT = CTRL // 128          # 4 ctrl tiles
NCHUNK = N_LOC // 128     # 512 location chunks
SUPER = 16                # chunks per DMA superblock
NSB = NCHUNK // SUPER     # 32 superblocks
GROUPS = [4, 4, 4, 4]     # exp batching: group 0 on VectorE, rest on ScalarE
DVE_EXP = True            # paired-Schraudolph exp on the Vector engine
LOG2E = 1.4426950408889634
SCH_A = 0.5 * LOG2E * (1 << 23)
SCH_C = 450000.0
SCH_B1 = 127.0 * (1 << 23) - (1 << 21) - SCH_C
SCH_B2 = 127.0 * (1 << 23) + (1 << 21) - SCH_C

_CACHE = {}
DEBUG_TAPS = False


def _build_program():
    import concourse.mybir as mybir
    import concourse.tile as tile
    from concourse import bacc
    from contextlib import ExitStack

    f32 = mybir.dt.float32
    bf16 = mybir.dt.bfloat16
    i32 = mybir.dt.int32
    AF = mybir.ActivationFunctionType
    AOT = mybir.AluOpType

    nc = bacc.Bacc("TRN2", target_bir_lowering=False, debug=False)

    # ---------------- DRAM I/O (per-core shapes; host pre-swizzled) ----------
    xTb = nc.dram_tensor("xTb", [128, KT, BC], bf16, kind="ExternalInput")
    xTl = nc.dram_tensor("xTl", [128, KT, BC], bf16, kind="ExternalInput")
    Whb = nc.dram_tensor("Whb", [128, KT, CTRL], bf16, kind="ExternalInput")
    Whl = nc.dram_tensor("Whl", [128, KT, CTRL], bf16, kind="ExternalInput")
    bh = nc.dram_tensor("bh", [128, CT], f32, kind="ExternalInput")
    Wgb = nc.dram_tensor("Wgb", [128, KT], bf16, kind="ExternalInput")
    Wgl = nc.dram_tensor("Wgl", [128, KT], bf16, kind="ExternalInput")
    bg = nc.dram_tensor("bg", [1, 1], f32, kind="ExternalInput")
    Wkb = nc.dram_tensor("Wkb", [128, KT, LOC], bf16, kind="ExternalInput")
    bk = nc.dram_tensor("bk", [LOC, 1], f32, kind="ExternalInput")
    MnT = nc.dram_tensor("MnT", [LOC, N_LOC], bf16, kind="ExternalInput")
    Mo = nc.dram_tensor("Mo", [128, NCHUNK, LOC + 1], bf16, kind="ExternalInput")
    WoH = nc.dram_tensor("WoH", [128, CT], f32, kind="ExternalInput")
    WoR = nc.dram_tensor("WoR", [LOC, 1], f32, kind="ExternalInput")
    bo = nc.dram_tensor("bo", [1, 1], f32, kind="ExternalInput")

    out_d = nc.dram_tensor("out", [1, BC], f32, kind="ExternalOutput")
    hl_d = nc.dram_tensor("hl", [128, CT], f32, kind="ExternalOutput")
    gt_d = nc.dram_tensor("gt", [1, BC], f32, kind="ExternalOutput")
    wl_d = nc.dram_tensor("wl", [128, NCHUNK], f32, kind="ExternalOutput")
    if DEBUG_TAPS:
        dbg_kn = nc.dram_tensor("dbg_kn", [LOC, BC], f32, kind="ExternalOutput")
        dbg_racc = nc.dram_tensor("dbg_racc", [LOC + 1, BC], f32,
                                  kind="ExternalOutput")

    with tile.TileContext(nc) as tc, ExitStack() as ctx:
        const = ctx.enter_context(tc.tile_pool(name="const", bufs=1))
        mnp = ctx.enter_context(tc.tile_pool(name="mnp", bufs=3))
        mop = ctx.enter_context(tc.tile_pool(name="mop", bufs=3))
        w2p = ctx.enter_context(tc.tile_pool(name="w2p", bufs=3))
        ps_sim = ctx.enter_context(tc.tile_pool(name="ps_sim", bufs=3, space="PSUM"))
        ps_r = ctx.enter_context(tc.tile_pool(name="ps_r", bufs=1, space="PSUM"))
        ps_misc = ctx.enter_context(tc.tile_pool(name="ps_misc", bufs=1, space="PSUM"))

        def tap(dram, ap, shape):
            t = const.tile(shape, f32, tag="tap" + dram.name)
            nc.vector.tensor_copy(out=t[:], in_=ap)
            nc.sync.dma_start(out=dram[:], in_=t[:])

        # ------------- load weights/inputs -------------
        xTb_sb = const.tile([128, KT, BC], bf16)
        nc.sync.dma_start(out=xTb_sb[:], in_=xTb[:])
        xTl_sb = const.tile([128, KT, BC], bf16)
        nc.sync.dma_start(out=xTl_sb[:], in_=xTl[:])
        Whb_sb = const.tile([128, KT, CTRL], bf16)
        nc.sync.dma_start(out=Whb_sb[:], in_=Whb[:])
        Whl_sb = const.tile([128, KT, CTRL], bf16)
        nc.sync.dma_start(out=Whl_sb[:], in_=Whl[:])
        bh_sb = const.tile([128, CT], f32)
        nc.sync.dma_start(out=bh_sb[:], in_=bh[:])
        Wgb_sb = const.tile([128, KT], bf16)
        nc.sync.dma_start(out=Wgb_sb[:], in_=Wgb[:])
        Wgl_sb = const.tile([128, KT], bf16)
        nc.sync.dma_start(out=Wgl_sb[:], in_=Wgl[:])
        bg_sb = const.tile([1, 1], f32)
        nc.sync.dma_start(out=bg_sb[:], in_=bg[:])
        Wkb_sb = const.tile([128, KT, LOC], bf16)
        nc.sync.dma_start(out=Wkb_sb[:], in_=Wkb[:])
        bk_sb = const.tile([LOC, 1], f32)
        nc.sync.dma_start(out=bk_sb[:], in_=bk[:])

        ones_sb = const.tile([128, 128], f32)
        nc.vector.memset(ones_sb[:], 1.0)

        # ------------- controller: h via bf16x2 (hi/lo) => ~fp32 accuracy ---
        hT_sb = const.tile([128, CT, BC], f32)     # for h[-1] export + out_h
        hTb_sb = const.tile([128, CT, BC], bf16)   # for the read-key matmul
        for ct in range(CT):
            ps_h = ps_sim.tile([128, BC], f32, tag="s")
            n = 0
            for k in range(KT):
                cs = slice(ct * 128, (ct + 1) * 128)
                for lhs, rhs in ((Whb_sb[:, k, cs], xTb_sb[:, k, :]),
                                 (Whl_sb[:, k, cs], xTb_sb[:, k, :]),
                                 (Whb_sb[:, k, cs], xTl_sb[:, k, :])):
                    nc.tensor.matmul(ps_h[:], lhs, rhs, start=(n == 0),
                                     stop=(n == 3 * KT - 1))
                    n += 1
            nc.scalar.activation(out=hT_sb[:, ct, :], in_=ps_h[:], func=AF.Tanh,
                                 bias=bh_sb[:, ct:ct + 1], scale=1.0)
            nc.scalar.activation(out=hTb_sb[:, ct, :], in_=ps_h[:], func=AF.Tanh,
                                 bias=bh_sb[:, ct:ct + 1], scale=1.0)

        # gate = x W_g + b_g (bf16x2)
        ps_g = ps_sim.tile([1, BC], f32, tag="s")
        n = 0
        for k in range(KT):
            for lhs, rhs in ((Wgb_sb[:, k:k + 1], xTb_sb[:, k, :]),
                             (Wgl_sb[:, k:k + 1], xTb_sb[:, k, :]),
                             (Wgb_sb[:, k:k + 1], xTl_sb[:, k, :])):
                nc.tensor.matmul(ps_g[:], lhs, rhs, start=(n == 0),
                                 stop=(n == 3 * KT - 1))
                n += 1
        gt_sb = const.tile([1, BC], f32)
        nc.scalar.activation(out=gt_sb[:], in_=ps_g[:], func=AF.Identity,
                             bias=bg_sb[0:1, 0:1], scale=1.0)
        nc.sync.dma_start(out=gt_d[:], in_=gt_sb[:])

        # h[-1]: column BC-1 of hT
        hl_sb = const.tile([128, CT], f32)
        nc.vector.tensor_copy(out=hl_sb[:], in_=hT_sb[:, :, BC - 1])
        nc.sync.dma_start(out=hl_d[:], in_=hl_sb[:])

        # out_h = hT^T WoH as [1, 256] -- independent of the streaming loop,
        # so compute it up front and stash in SBUF (keeps the tail short)
        WoH_sb = const.tile([128, CT], f32)
        nc.sync.dma_start(out=WoH_sb[:], in_=WoH[:])
        WoR_sb = const.tile([LOC, 1], f32)
        nc.sync.dma_start(out=WoR_sb[:], in_=WoR[:])
        bo_sb = const.tile([1, 1], f32)
        nc.sync.dma_start(out=bo_sb[:], in_=bo[:])
        ps_oh = ps_misc.tile([1, BC], f32, tag="m")
        for k in range(CT):
            nc.tensor.matmul(ps_oh[:], WoH_sb[:, k:k + 1], hT_sb[:, k, :],
                             start=(k == 0), stop=(k == CT - 1))
        oh_sb = const.tile([1, BC], f32)
        nc.vector.tensor_copy(out=oh_sb[:], in_=ps_oh[:])

        ps_rk = ps_sim.tile([LOC, BC], f32, tag="s")
        for k in range(KT):
            nc.tensor.matmul(ps_rk[:], Wkb_sb[:, k, :], hTb_sb[:, k, :],
                             start=(k == 0), stop=(k == KT - 1))
        rkT_sb = const.tile([LOC, BC], bf16)
        nc.scalar.activation(out=rkT_sb[:], in_=ps_rk[:], func=AF.Tanh,
                             bias=bk_sb[:], scale=1.0)

        # kn = rk / ||rk||: sum of squares via ones-matmul, rsqrt via ln/exp
        rksq_sb = const.tile([LOC, BC], f32)
        nc.vector.tensor_mul(rksq_sb[:], rkT_sb[:], rkT_sb[:])
        ps_ss = ps_sim.tile([1, BC], f32, tag="s")
        nc.tensor.matmul(ps_ss[:], ones_sb[0:LOC, 0:1], rksq_sb[:],
                         start=True, stop=True)
        lnss_sb = const.tile([1, BC], f32)
        nc.scalar.activation(out=lnss_sb[:], in_=ps_ss[:], func=AF.Ln)
        invn_sb = const.tile([1, BC], f32)
        nc.scalar.activation(out=invn_sb[:], in_=lnss_sb[:], func=AF.Exp,
                             scale=-0.5)
        ps_bc = ps_sim.tile([LOC, BC], f32, tag="s")
        nc.tensor.matmul(ps_bc[:], ones_sb[0:1, 0:LOC], invn_sb[:],
                         start=True, stop=True)
        knT_sb = const.tile([LOC, BC], bf16)
        nc.vector.tensor_mul(knT_sb[:], rkT_sb[:], ps_bc[:])
        if DEBUG_TAPS:
            tap(dbg_kn, knT_sb[:], [LOC, BC])

        # ------------- streaming pass over the 65536 memory locations -------
        wlast_sb = const.tile([128, NCHUNK], f32)   # unnormalized w of sample BC-1
        ps_rT = ps_r.tile([LOC + 1, BC], f32)       # rows 0..63: r^T; row 64: Z

        r_count = [0]

        def emit_r(mo_tile, w2_tile, base, g0, gsz):
            # start/stop must follow EMISSION (=execution) order, not chunk
            # index: a late start=True would wipe prior accumulation.
            for j in range(g0, g0 + gsz):
                nc.tensor.matmul(ps_rT[:], mo_tile[:, j, :], w2_tile[:, j, :],
                                 start=(r_count[0] == 0),
                                 stop=(r_count[0] == NCHUNK - 1))
                r_count[0] += 1

        G = GROUPS[0]
        NG = SUPER // G                        # groups per superblock
        pending = None   # DVE-exp'd group of the previous superblock
        for sb in range(NSB):
            c0 = sb * SUPER
            mn_t = mnp.tile([LOC, SUPER, 128], bf16)
            nc.sync.dma_start(
                out=mn_t[:],
                in_=MnT[:, c0 * 128:(c0 + SUPER) * 128].rearrange(
                    "f (s c) -> f s c", s=SUPER),
            )
            mo_t = mop.tile([128, SUPER, LOC + 1], bf16)
            nc.sync.dma_start(out=mo_t[:], in_=Mo[:, c0:c0 + SUPER, :])
            w2_t = w2p.tile([128, SUPER, BC], bf16)

            # group 0: sims then exp on the Vector engine (max slack: its
            # r-matmuls are deferred to the next superblock)
            ps_s0 = ps_sim.tile([128, G, BC], f32, tag="s")
            for j in range(G):
                nc.tensor.matmul(ps_s0[:, j, :], mn_t[:, j, :], knT_sb[:],
                                 start=True, stop=True)
            if DVE_EXP:
                # exp via product of two phase-shifted Schraudolph
                # approximations: bitcast(int32(x*A+B)) ~ 2^(x*log2e/2)
                y1_t = w2p.tile([128, G, BC], i32, tag="y1")
                nc.vector.tensor_scalar(out=y1_t[:], in0=ps_s0[:],
                                        scalar1=SCH_A, scalar2=SCH_B1,
                                        op0=AOT.mult, op1=AOT.add)
                y2_t = w2p.tile([128, G, BC], i32, tag="y2")
                nc.vector.tensor_scalar(out=y2_t[:], in0=ps_s0[:],
                                        scalar1=SCH_A, scalar2=SCH_B2,
                                        op0=AOT.mult, op1=AOT.add)
                nc.gpsimd.tensor_mul(w2_t[:, 0:G, :],
                                      y1_t[:].bitcast(f32), y2_t[:].bitcast(f32))
            else:
                nc.scalar.activation(out=w2_t[:, 0:G, :], in_=ps_s0[:],
                                     func=AF.Exp)
            # groups 1..NG-1: sims + ACT exp, r-matmuls interleaved
            for gi in range(1, NG):
                g0 = gi * G
                ps_s = ps_sim.tile([128, G, BC], f32, tag="s")
                for j in range(G):
                    nc.tensor.matmul(ps_s[:, j, :], mn_t[:, g0 + j, :],
                                     knT_sb[:], start=True, stop=True)
                nc.scalar.activation(out=w2_t[:, g0:g0 + G, :], in_=ps_s[:],
                                     func=AF.Exp)
                if gi == 1:
                    if pending is not None:
                        emit_r(*pending)
                        pending = None
                    if not DVE_EXP:
                        emit_r(mo_t, w2_t, c0, 0, G)
                else:
                    emit_r(mo_t, w2_t, c0, (gi - 1) * G, G)
            emit_r(mo_t, w2_t, c0, (NG - 1) * G, G)
            if DVE_EXP:
                pending = (mo_t, w2_t, c0, 0, G)
            nc.gpsimd.tensor_copy(out=wlast_sb[:, c0:c0 + SUPER],
                                  in_=w2_t[:, :, BC - 1])
        if pending is not None:
            emit_r(*pending)
            pending = None
        if DEBUG_TAPS:
            tap(dbg_racc, ps_rT[:], [LOC + 1, BC])

        # ------------- epilogue ---------------------------------------------
        # 1/Z on partition 64 (where Z landed) via exp(-ln(Z)) on ScalarE
        lnz_sb = const.tile([128, BC], f32)
        nc.scalar.activation(out=lnz_sb[64:65, :], in_=ps_rT[LOC:LOC + 1, :],
                             func=AF.Ln)
        zw_sb = const.tile([128, BC], f32)
        nc.scalar.activation(out=zw_sb[64:65, :], in_=lnz_sb[64:65, :],
                             func=AF.Exp, scale=-1.0)

        racc_sb = const.tile([LOC, BC], f32)
        nc.vector.tensor_copy(out=racc_sb[:], in_=ps_rT[0:LOC, :])

        # broadcast 1/Z from partition 64 to partition 0 (row 0 of [64, BC])
        ps_zb = ps_misc.tile([LOC, BC], f32, tag="m")
        nc.tensor.matmul(ps_zb[:], ones_sb[64:65, 0:LOC], zw_sb[64:65, :],
                         start=True, stop=True)
        zb_sb = const.tile([LOC, BC], f32)
        nc.vector.tensor_copy(out=zb_sb[:], in_=ps_zb[:])

        # out = out_h + (rT^T WoR) / Z + bo as [1, 256]
        ps_or = ps_misc.tile([1, BC], f32, tag="m")
        nc.tensor.matmul(ps_or[:], WoR_sb[:], racc_sb[:], start=True, stop=True)
        t1_sb = const.tile([1, BC], f32)
        nc.vector.tensor_mul(t1_sb[:], zb_sb[0:1, :], ps_or[:])
        t2_sb = const.tile([1, BC], f32)
        nc.vector.tensor_add(t2_sb[:], t1_sb[:], oh_sb[:])
        outv_sb = const.tile([1, BC], f32)
        nc.vector.tensor_scalar_add(outv_sb[:], t2_sb[:], bo_sb[0:1, 0:1])
        nc.sync.dma_start(out=out_d[:], in_=outv_sb[:])

        # w_read[-1] = wlast * (1/Z[BC-1]) broadcast to all 128 partitions
        ps_zl = ps_misc.tile([128, 1], f32, tag="m")
        nc.tensor.matmul(ps_zl[:], ones_sb[64:65, :], zw_sb[64:65, BC - 1:BC],
                         start=True, stop=True)
        zl_sb = const.tile([128, 1], f32)
        nc.vector.tensor_copy(out=zl_sb[:], in_=ps_zl[:])
        wlf_sb = const.tile([128, NCHUNK], f32)
        nc.vector.tensor_scalar_mul(wlf_sb[:], wlast_sb[:], zl_sb[:, 0:1])
        nc.sync.dma_start(out=wl_d[:], in_=wlf_sb[:])

    nc.compile()
    return nc


def _get_program():
    if "nc" not in _CACHE:
        _CACHE["nc"] = _build_program()
    return _CACHE["nc"]


def _prep_in_maps(inputs):
    return _prep(**{k: np.asarray(v) for k, v in inputs.items()})


def _prep(x, W_h, b_h, W_g, b_g, W_k, b_k, M, W_out, b_out):
    import ml_dtypes
    bf = ml_dtypes.bfloat16

    x = np.ascontiguousarray(np.asarray(x, dtype=np.float32))
    W_h = np.asarray(W_h, dtype=np.float32)
    b_h = np.asarray(b_h, dtype=np.float32)
    W_g = np.asarray(W_g, dtype=np.float32)
    b_g = np.asarray(b_g, dtype=np.float32)
    W_k = np.asarray(W_k, dtype=np.float32)
    b_k = np.asarray(b_k, dtype=np.float32)
    M = np.ascontiguousarray(np.asarray(M, dtype=np.float32))
    W_out = np.asarray(W_out, dtype=np.float32)
    b_out = np.asarray(b_out, dtype=np.float32)

    # ---- host-side layout prep (weight swizzles for SBUF-friendly DMA) ----
    norms = np.linalg.norm(M, axis=1, keepdims=True)
    MnT = np.ascontiguousarray((M / (norms + EPS)).T).astype(bf)  # [64, 65536]
    Mo = np.concatenate([M, np.ones((N_LOC, 1), np.float32)], axis=1)
    Mo = np.ascontiguousarray(
        Mo.reshape(NCHUNK, 128, LOC + 1).transpose(1, 0, 2)).astype(bf)

    Wh_p = np.ascontiguousarray(W_h.reshape(KT, 128, CTRL).transpose(1, 0, 2))
    bh_p = np.ascontiguousarray(b_h.reshape(CT, 128).T)
    Wg_p = np.ascontiguousarray(W_g[:, 0].reshape(KT, 128).T)
    bg_p = b_g.reshape(1, 1)
    Wk_p = np.ascontiguousarray(W_k.reshape(KT, 128, LOC).transpose(1, 0, 2))
    bk_p = b_k.reshape(LOC, 1)
    WoH_p = np.ascontiguousarray(W_out[:CTRL, 0].reshape(CT, 128).T)
    WoR_p = np.ascontiguousarray(W_out[CTRL:, 0:1])
    bo_p = b_out.reshape(1, 1)

    def hilo(a):
        hi = a.astype(bf)
        lo = (a - hi.astype(np.float32)).astype(bf)
        return hi, lo

    Whb_p, Whl_p = hilo(Wh_p)
    Wgb_p, Wgl_p = hilo(Wg_p)
    shared = dict(Whb=Whb_p, Whl=Whl_p, bh=bh_p, Wgb=Wgb_p, Wgl=Wgl_p, bg=bg_p,
                  Wkb=Wk_p.astype(bf), bk=bk_p, MnT=MnT, Mo=Mo,
                  WoH=WoH_p, WoR=WoR_p, bo=bo_p)
    in_maps = []
    for c in range(NCORES):
        xc = x[c * BC:(c + 1) * BC]                              # [256, 512]
        xT_p = np.ascontiguousarray(
            xc.T.reshape(KT, 128, BC).transpose(1, 0, 2))        # [128, 4, 256]
        xb, xl = hilo(xT_p)
        in_maps.append(dict(shared, xTb=xb, xTl=xl))
    return in_maps


def kernel(x, W_h, b_h, W_g, b_g, W_k, b_k, M, W_out, b_out):
    from concourse.bass_utils import run_bass_kernel_spmd

    in_maps = _prep(x, W_h, b_h, W_g, b_g, W_k, b_k, M, W_out, b_out)
    nc = _get_program()
    res = run_bass_kernel_spmd(nc, in_maps, core_ids=list(range(NCORES))).results

    output = np.concatenate([res[c]["out"][0] for c in range(NCORES)])
    h_last = np.ascontiguousarray(res[NCORES - 1]["hl"].T).reshape(CTRL)
    gate_last = res[NCORES - 1]["gt"][0, BC - 1:BC].copy()
    w_read_last = np.ascontiguousarray(res[NCORES - 1]["wl"].T).reshape(N_LOC)
    return (output.astype(np.float32), h_last.astype(np.float32),
            gate_last.astype(np.float32), w_read_last.astype(np.float32))
